# revision 1
# baseline (speedup 1.0000x reference)
"""Bass program builder for the CLAHE augmenter (one SPMD program per core-shape).

Slots per core: KC CLAHE + KP passthrough. Serial stage chain via one `seq`
semaphore (correctness-first; engines idle between their stages).
"""
import sys
sys.path.insert(0, '/opt/trn_rl_repo')
import numpy as np
import ml_dtypes
import concourse.bass as bass
import concourse.mybir as mybir

F32 = np.float32
BF16 = ml_dtypes.bfloat16
A = mybir.AluOpType
ACTF = mybir.ActivationFunctionType
CROP = 512

def tx_centers():
    t = (np.arange(CROP, dtype=np.float32) + F32(0.5)) / F32(64.0) - F32(0.5)
    t0 = np.floor(t)
    f = (t - t0).astype(np.float32)
    i0 = np.clip(t0, 0, 7).astype(np.int32)
    i1 = np.clip(t0 + 1, 0, 7).astype(np.int32)
    return i0, i1, f

# fetch row ranges: (row0, nrows, qa, qb) with qa/qb the two y-bands
def row_ranges():
    rr = [(0, 32, 0, 0)]
    for q in range(7):
        rr.append((32 + 64 * q, 64, q, q + 1))
    rr.append((480, 32, 7, 7))
    return rr

def make_consts():
    c = {}
    c['iota32'] = np.broadcast_to(np.arange(32, dtype=np.float32), (128, 32)).copy()
    j = np.arange(16, dtype=np.float32)[None, :].repeat(128, 0).copy()
    j[64:] -= 8.0
    c['wext16'] = j.astype(np.float32)
    c['identbf'] = np.eye(128, dtype=np.float32).astype(BF16)
    c['identf32'] = np.eye(128, dtype=np.float32)
    c['ident16'] = np.eye(16, dtype=np.float32)
    c['ident4'] = ((np.arange(128)[:, None] % 4) == np.arange(4)[None, :]).astype(np.float32)
    p = np.arange(128)
    c['feedconst'] = ((p % 32).astype(np.float32)).reshape(128, 1)
    # rep selector consts: variant v selects cols 4v..4v+3 of a 128-col batch
    rep = np.zeros((128, 32, 128), np.float32)
    for v in range(32):
        for ci in range(4):
            for j in range(32):
                rep[4*v + ci, v, ci*32 + j] = 1.0
    c['repconst'] = rep.reshape(128, 32*128).astype(BF16)
    # R partition layout: k = c*32 + tr*16 + side*8 + lo
    c['loconst'] = ((p & 7).astype(np.float32)).reshape(128, 1)
    tri = np.zeros((16, 16), np.float32)   # lhsT[k, m] = 1 iff same tile & lo(k) < lo(m)
    ones = np.zeros((16, 16), np.float32)
    for t in range(2):
        tri[t*8:(t+1)*8, t*8:(t+1)*8] = np.triu(np.ones((8, 8)), 1)
        ones[t*8:(t+1)*8, t*8:(t+1)*8] = 1.0
    c['tri16'] = tri
    c['ones16'] = ones
    lo_p = (np.arange(16) % 8)[:, None]
    hi_f = np.arange(32)[None, :]
    c['binconst'] = (lo_p * 32 + hi_f + 1).astype(np.float32)
    # S2 per group: [NG, 128 K, 4 M] bf16;  K = c*32 + tr*16 + side*8 + lo
    x0, x1, fx = tx_centers()
    NG = 128
    S2t = np.zeros((NG, 128, 4), np.float32)
    S2b = np.zeros((NG, 128, 4), np.float32)
    inv = 1.0 / 255.0
    for g in range(NG):
        for ci in range(4):
            wx = fx[4 * g + ci]
            for side in range(2):
                w = (1.0 - wx) if side == 0 else wx
                for lo in range(8):
                    S2t[g, ci*32 + 0*16 + side*8 + lo, ci] = w * inv
                    S2b[g, ci*32 + 1*16 + side*8 + lo, ci] = w * inv
    c['s2top'] = np.ascontiguousarray(S2t.transpose(1, 0, 2).reshape(128, NG * 4)).astype(np.float16)
    c['s2bot'] = np.ascontiguousarray(S2b.transpose(1, 0, 2).reshape(128, NG * 4)).astype(np.float16)
    _, _, fy = tx_centers()
    c['fyconst'] = np.broadcast_to(fy, (128, CROP)).astype(np.float32).copy()
    return c

def const_arrays():
    return make_consts()

def build(KC, KP):
    consts = make_consts()
    NS = KC + KP
    nc = bass.Bass("TRN2", target_bir_lowering=False)
    xs = nc.dram_tensor("xs", [NS, 640, 640], mybir.dt.float32, kind="ExternalInput")
    meta = nc.dram_tensor("meta", [NS, 4], mybir.dt.int32, kind="ExternalInput")
    out = nc.dram_tensor("out", [NS, 512, 512], mybir.dt.float32, kind="ExternalOutput")
    cdram = {}
    for name, v in consts.items():
        dt = {np.dtype(np.float32): mybir.dt.float32,
              np.dtype(np.float16): mybir.dt.float16,
              np.dtype(BF16): mybir.dt.bfloat16}[v.dtype]
        cdram[name] = nc.dram_tensor("c_" + name, [v.shape[0], int(np.prod(v.shape[1:]))], dt,
                                     kind="ExternalInput")

    x0c, x1c, _ = tx_centers()
    RR = row_ranges()

    # ---------------- SBUF ----------------
    sb = {}
    def sbuf(name, shape, dt):
        sb[name] = nc.alloc_sbuf_tensor(name, shape, dt)
        return sb[name]

    cs = {}
    for name, v in consts.items():
        dt = {np.dtype(np.float32): mybir.dt.float32,
              np.dtype(np.float16): mybir.dt.float16,
              np.dtype(BF16): mybir.dt.bfloat16}[v.dtype]
        cs[name] = nc.alloc_sbuf_tensor("s_" + name, [v.shape[0], int(np.prod(v.shape[1:]))], dt)

    metas = sbuf("metas", [1, NS * 4], mybir.dt.int32)
    xfull = sbuf("xfull", [128, 2048], mybir.dt.float32)
    tA = sbuf("tA", [128, 512], mybir.dt.float32)     # t256 / reused
    tB = sbuf("tB", [128, 512], mybir.dt.float32)     # u/r256
    tC = sbuf("tC", [128, 512], mybir.dt.float32)     # t8
    tD = sbuf("tD", [128, 512], mybir.dt.float32)     # u/r8
    tE = sbuf("tE", [128, 512], mybir.dt.float32)     # scratch (gt/b)
    tE2 = sbuf("tE2", [128, 512], mybir.dt.float32)   # scratch
    hibf = sbuf("hibf", [128, 4 * 512], mybir.dt.float32)  # per slab kept
    lobf = sbuf("lobf", [128, 4 * 512], mybir.dt.float32)
    u32 = sbuf("u32", [128, 512 * 32], mybir.dt.bfloat16)
    w16 = sbuf("w16", [128, 512 * 16], mybir.dt.bfloat16)
    lutrep = sbuf("lutrep", [16, 1024], mybir.dt.float32)
    master2 = sbuf("master2", [128, 81 * 128], mybir.dt.float16)
    hiT = sbuf("hiT", [128, 2048], mybir.dt.bfloat16)
    loT = sbuf("loT", [128, 2048], mybir.dt.bfloat16)
    hists = sbuf("hists", [16, 4 * 256], mybir.dt.float32)
    mbuf = sbuf("mbuf", [16, 256], mybir.dt.float32)
    rowtot = sbuf("rowtot", [16, 8], mybir.dt.float32)
    gsb = sbuf("gsb", [16, 8], mybir.dt.float32)
    e8 = sbuf("e8", [16, 8], mybir.dt.float32)
    ccb = sbuf("ccb", [16, 256], mybir.dt.float32)
    lutsb = sbuf("lutsb", [16, 256], mybir.dt.float32)
    master = sbuf("master", [128, 2048], mybir.dt.float16)  # (tr8, tc8, lo8, c4)
    u32t = [sbuf("u32t0", [128, 512], mybir.dt.float16), sbuf("u32t1", [128, 512], mybir.dt.float16)]
    w32tsb = [sbuf("w32tsb0", [128, 512], mybir.dt.float32), sbuf("w32tsb1", [128, 512], mybir.dt.float32)]
    mrk = [sbuf("mrk0", [128, 512], mybir.dt.float16), sbuf("mrk1", [128, 512], mybir.dt.float16)]
    topsb = sbuf("topsb", [128, 512], mybir.dt.float32)
    vg = sbuf("vg", [4, 512], mybir.dt.float32)
    vg2 = sbuf("vg2", [4, 512], mybir.dt.float32)
    vbat = sbuf("vbat", [128, 512], mybir.dt.float32)
    vbat2 = sbuf("vbat2", [128, 512], mybir.dt.float32)
    top4 = sbuf("top4", [4, 512], mybir.dt.float32)
    ostg = sbuf("ostg", [128, 2048], mybir.dt.float32)   # (row128, slab4*col512)
    ostage = sbuf("ostage", [128, 512], mybir.dt.float32)  # PT staging
    hstage = sbuf("hstage", [128, 2048], mybir.dt.float32)
    outb = sbuf("outb", [128, 2048], mybir.dt.float32)
    ctmp = sbuf("ctmp", [128, 512], mybir.dt.float32)

    # ---------------- PSUM ----------------
    ps = {}
    def psum(name, shape, dt):
        ps[name] = nc.alloc_psum_tensor(name, shape, dt)
        return ps[name]
    pH = psum("pH", [16, 256], mybir.dt.float32)
    pGE = psum("pGE", [16, 16], mybir.dt.float32)
    pT = psum("pT", [128, 128], mybir.dt.float32)
    pHT = psum("pHT", [128, 512], mybir.dt.float32)
    pLT = psum("pLT", [128, 512], mybir.dt.float32)
    pR = psum("pR", [128, 512], mybir.dt.float32)
    pTOP = psum("pTOP", [128, 512], mybir.dt.float32)
    pBOT = psum("pBOT", [128, 512], mybir.dt.float32)
    pOT = pT

    # ------------- stage machinery -------------
    # collected as (engine_name, fn) in global order; emitted per engine with
    # wait_ge(seq, idx) before and inc after.
    stages = []
    def stage(eng):
        def deco(fn):
            stages.append((eng, fn))
            return fn
        return deco

    TWO23 = float(2 ** 23)

    def emit_clahe(i):
        # ---- dynamic crop DMA (4 slabs loaded one at a time inside pass1) ----
        # offsets into regs on sync engine; recomputed per slab stage (simple).
        def load_offsets(sync, s):
            oy = sync.alloc_register(f"oy{i}_{s}")
            ox = sync.alloc_register(f"ox{i}_{s}")
            hf = sync.alloc_register(f"hf{i}_{s}")
            vf = sync.alloc_register(f"vf{i}_{s}")
            sync.reg_load(oy, metas.ap()[:1, 4*i:4*i+1])
            sync.reg_load(ox, metas.ap()[:1, 4*i+1:4*i+2])
            sync.reg_load(hf, metas.ap()[:1, 4*i+2:4*i+3])
            sync.reg_load(vf, metas.ap()[:1, 4*i+3:4*i+4])
            return oy, ox, hf, vf

        @stage("sync")
        def _(sync, i=i):
            oy, ox, hf, vf = load_offsets(sync, 0)
            t1 = sync.alloc_register(f"t1_{i}")
            with sync.If_cmp(vf, 0, "IS_NE"):
                sync.reg_mov(t1, 128)
                sync.reg_alu(t1, t1, oy, A.subtract)
                sync.reg_mov(oy, t1)
            sync.end_ifs()
            with sync.If_cmp(hf, 0, "IS_NE"):
                sync.reg_mov(t1, 128)
                sync.reg_alu(t1, t1, ox, A.subtract)
                sync.reg_mov(ox, t1)
            sync.end_ifs()
            soy = sync.snap(oy, donate=True)
            sox = sync.snap(ox, donate=True)
            d = nc.alloc_semaphore(f"d_cl_{i}")
            src2d = xs[i][bass.ds(soy, 128), bass.ds(sox, 512)]
            src3d = bass.AP(src2d.tensor, src2d.offset,
                            [[640, 128], [128*640, 4], [1, 512]],
                            runtime_checks=src2d.runtime_checks)
            sync.dma_start(bass.AP(xfull, 0, [[2048, 128], [512, 4], [1, 512]]),
                           src3d).then_inc(d, 16)
            sync.wait_ge(d, 16)
            for r in (oy, ox, hf, vf, t1):
                sync.free_register(r)
            try:
                sync.get_value_cache().clear()
            except Exception:
                pass

        # pass 1 per slab
        for s in range(4):
            @stage("scalar")
            def _(act, s=s, i=i):
                hfr = act.alloc_register(f"ahf{i}_{s}")
                act.reg_load(hfr, metas.ap()[:1, 4*i+2:4*i+3])
                def emit(x):
                    act.activation(tA[:, :], x, ACTF.Copy, bias=0.0, scale=256.0)
                    act.activation(tB[:, :], x, ACTF.Copy, bias=TWO23, scale=256.0)
                    act.activation(tC[:, :], x, ACTF.Copy, bias=0.0, scale=8.0)
                    act.activation(tD[:, :], x, ACTF.Copy, bias=TWO23, scale=8.0)
                    act.drain()
                    act.activation(tB[:, :], tB[:, :], ACTF.Copy, bias=-TWO23, scale=1.0)
                    act.activation(tD[:, :], tD[:, :], ACTF.Copy, bias=-TWO23, scale=1.0)
                with act.If_cmp(hfr, 0, "IS_EQ"):
                    emit(xfull.ap()[:, 512*s:512*(s+1)])
                with act.Else():
                    emit(xfull.ap()[:, 512*s:512*(s+1)][:, ::-1])
                act.end_ifs()
                act.free_register(hfr)

            @stage("vector")
            def _(v, s=s):
                # b = r256 - (r256 > t256); lo = r8 - (r8 > t8); hi = b - 32*lo
                v.tensor_tensor(tE[:, :], tB[:, :], tA[:, :], A.is_gt)
                v.drain()
                v.tensor_tensor(tA[:, :], tB[:, :], tE[:, :], A.subtract)   # tA = b
                v.tensor_tensor(tE2[:, :], tD[:, :], tC[:, :], A.is_gt)
                v.drain()
                v.tensor_tensor(tC[:, :], tD[:, :], tE2[:, :], A.subtract)  # tC = lo
                v.drain()
                v.scalar_tensor_tensor(tB[:, :], tC[:, :], -32.0, tA[:, :], A.mult, A.add)  # tB = hi
                v.drain()
                v.tensor_copy(hibf.ap()[:, 512*s:512*(s+1)], tB[:, :])
                v.tensor_copy(lobf.ap()[:, 512*s:512*(s+1)], tC[:, :])

            @stage("vector")
            def _(v, s=s):
                hb = hibf.ap()[:, 512*s:512*(s+1)]
                lb = lobf.ap()[:, 512*s:512*(s+1)]
                io32 = cs['iota32'].ap()
                we = cs['wext16'].ap()
                v.tensor_tensor(
                    bass.AP(u32, 0, [[512*32, 128], [32, 512], [1, 32]]),
                    hb[:, :, None].broadcast_to([128, 512, 32]),
                    io32[:, None, :].broadcast_to([128, 512, 32]),
                    A.is_equal)
                v.tensor_tensor(
                    bass.AP(w16, 0, [[512*16, 128], [16, 512], [1, 16]]),
                    lb[:, :, None].broadcast_to([128, 512, 16]),
                    we[:, None, :].broadcast_to([128, 512, 16]),
                    A.is_equal)

            @stage("tensor")
            def _(t, s=s):
                ins = None
                for tc in range(8):
                    for col in range(64):
                        cc = tc * 64 + col
                        ins = t.matmul(pH.ap()[:, tc*32:(tc+1)*32],
                                 bass.AP(w16, cc*16, [[512*16, 128], [1, 16]]),
                                 bass.AP(u32, cc*32, [[512*32, 128], [1, 32]]),
                                 start=(col == 0), stop=(col == 63))
                return ins

            @stage("vector")
            def _(v, s=s):
                return v.tensor_copy(hists.ap()[:16, 256*s:256*(s+1)], pH.ap()[:, :])

        # ---- LUT build + master ----
        @stage("gpsimd")
        def _(g):
            g.memset(master.ap()[:, :], 0)

        for s in range(4):
            @stage("vector")
            def _(v, s=s):
                h = hists.ap()[:16, 256*s:256*(s+1)]
                v.tensor_scalar(mbuf.ap()[:, :], h, 12.8, None, A.min)
                v.drain()
                v.tensor_reduce(rowtot.ap()[:, :],
                                bass.AP(mbuf, 0, [[256, 16], [32, 8], [1, 32]]),
                                mybir.AxisListType.X, A.add)

            @stage("tensor")
            def _(t, s=s):
                t.matmul(pGE.ap()[:, 0:8], cs['tri16'].ap()[:16, :], rowtot.ap()[:, :],
                         start=True, stop=True)
                return t.matmul(pGE.ap()[:, 8:16], cs['ones16'].ap()[:16, :], rowtot.ap()[:, :],
                         start=True, stop=True)

            @stage("vector")
            def _(v, s=s):
                v.tensor_copy(gsb.ap()[:, :], pGE.ap()[:, 0:8])
                v.tensor_scalar(e8.ap()[:, :], pGE.ap()[:, 8:16], 4096.0, -1.0/256.0,
                                A.subtract, A.mult, )
                # e8 = (S_all - 4096) * (-1/256) = (4096 - S_all)/256
                for tc in range(8):
                    v.tensor_tensor_scan(
                        ccb.ap()[:, tc*32:(tc+1)*32],
                        mbuf.ap()[:, tc*32:(tc+1)*32],
                        mbuf.ap()[:, tc*32:(tc+1)*32],
                        0.0, A.add, A.bypass)
                cc3 = bass.AP(ccb, 0, [[256, 16], [32, 8], [1, 32]])
                mb3 = bass.AP(mbuf, 0, [[256, 16], [32, 8], [1, 32]])
                v.drain()
                # mbuf = CC = rowpref + G  (mbuf re-used as CC)
                v.tensor_tensor(mb3, cc3,
                                gsb.ap()[:, :, None].broadcast_to([16, 8, 32]),
                                A.add)
                # ccb = binconst * e8
                lut3 = bass.AP(lutsb, 0, [[256, 16], [32, 8], [1, 32]])
                v.tensor_tensor(lut3,
                                cs['binconst'].ap()[:16, None, :].broadcast_to([16, 8, 32]),
                                e8.ap()[:, :, None].broadcast_to([16, 8, 32]),
                                A.mult)
                v.drain()
                v.tensor_tensor(ccb.ap()[:, :], lutsb.ap()[:, :], mbuf.ap()[:, :], A.add)
                v.drain()
                v.tensor_scalar(lutsb.ap()[:, :], ccb.ap()[:, :], float(255.0/4096.0), 255.0,
                                A.mult, A.min)

            @stage("vector")
            def _(v, s=s):
                # materialize rep: lutrep[16, (tc8, rep4, hi32)] (single-dim for PE)
                return v.tensor_copy(bass.AP(lutrep, 0, [[1024, 16], [128, 8], [32, 4], [1, 32]]),
                              bass.AP(lutsb, 0, [[256, 16], [32, 8], [0, 4], [1, 32]]))

            @stage("tensor")
            def _(t, s=s):
                ins = None
                for tc in range(8):
                    ins = t.transpose(pT.ap()[:, tc*16:(tc+1)*16],
                                lutrep.ap()[:, tc*128:(tc+1)*128],
                                cs['ident16'].ap()[:16, :16])
                return ins

            @stage("vector")
            def _(v, s=s):
                # master copies: for c: pT[32c:32c+32, (tc8,tAB2,lo8)] -> master
                # master f = tr*256 + tc*32 + lo*4 + c
                ins = None
                for c in range(4):
                    inap = bass.AP(pT, 32*c*128, [[128, 32], [8, 2], [16, 8], [1, 8]])
                    outap = bass.AP(master, 32*c*2048 + s*2*256 + c, [[2048, 32], [256, 2], [32, 8], [4, 8]])
                    ins = v.tensor_copy(outap, inap)
                return ins

        # ---- master2: contiguous per-(trpair, xspan) blocks ----
        TRP = [(0, 0)] + [(q, q + 1) for q in range(7)] + [(7, 7)]
        XSP = [(0, 0)] + [(k, k + 1) for k in range(7)] + [(7, 7)]
        @stage("vector")
        def _(v):
            ins = None
            for bi, (qa, qb) in enumerate(TRP):
                for xi, (xa, xb) in enumerate(XSP):
                    blk_off = (bi * 9 + xi) * 128
                    inap = bass.AP(master, qa*256 + xa*32,
                                   [[2048, 128], [1, 4], [(qb-qa)*256, 2],
                                    [(xb-xa)*32, 2], [4, 8]])
                    outap = bass.AP(master2, blk_off,
                                    [[81*128, 128], [32, 4], [16, 2], [8, 2], [1, 8]])
                    ins = v.tensor_copy(outap, inap)
            return ins

        # ---- pass 2 prologue: hiT/loT (transposed raw keys) ----
        for b in range(4):
            for s in range(4):
                @stage("tensor")
                def _(t, b=b, s=s):
                    return t.transpose(pT.ap()[:, :],
                                hibf.ap()[:, 512*s + 128*b: 512*s + 128*(b+1)],
                                cs['identf32'].ap()[:, :])

                @stage("vector")
                def _(v, b=b, s=s):
                    return v.tensor_copy(hiT.ap()[:, (b*4+s)*128:(b*4+s+1)*128], pT.ap()[:, :])

                @stage("tensor")
                def _(t, b=b, s=s):
                    return t.transpose(pT.ap()[:, :],
                                lobf.ap()[:, 512*s + 128*b: 512*s + 128*(b+1)],
                                cs['identf32'].ap()[:, :])

                @stage("vector")
                def _(v, b=b, s=s):
                    return v.tensor_copy(loT.ap()[:, (b*4+s)*128:(b*4+s+1)*128], pT.ap()[:, :])

        # ---- pass 2: fetch/blend per group ----
        @stage("vector")
        def _(v):
            v.memset(pTOP.ap()[:, :], 0.0)
            return v.memset(pBOT.ap()[:, :], 0.0)

        for g in range(128):
            bb = g // 32
            vv = g % 32

            @stage("tensor")
            def _(t, g=g, bb=bb, vv=vv):
                rc = cs['repconst'].ap()[:, vv*128:(vv+1)*128]
                for s in range(4):
                    t.matmul(pHT.ap()[:, 128*s:128*(s+1)], rc,
                             hiT.ap()[:, (bb*4+s)*128:(bb*4+s+1)*128],
                             start=True, stop=True)
                    t.matmul(pLT.ap()[:, 128*s:128*(s+1)], rc,
                             loT.ap()[:, (bb*4+s)*128:(bb*4+s+1)*128],
                             start=True, stop=True)

            @stage("vector")
            def _(v, g=g):
                v.tensor_tensor(u32t[g % 2].ap()[:, :], pHT.ap()[:, :],
                                cs['feedconst'].ap()[:, 0:1].broadcast_to([128, 512]),
                                A.is_equal)
                return v.tensor_tensor(w32tsb[g % 2].ap()[:, :], pLT.ap()[:, :],
                                cs['loconst'].ap()[:, 0:1].broadcast_to([128, 512]),
                                A.is_equal)

            @stage("tensor")
            def _(t, g=g):
                xi = 0 if g < 8 else (8 if g >= 120 else 1 + (g - 8) // 16)
                ins = None
                for rrid, (r0, nr, qa, qb) in enumerate(row_ranges()):
                    blk_off = (rrid * 9 + xi) * 128
                    ins = t.matmul(pR.ap()[:, r0:r0+nr],
                             master2.ap()[:, blk_off:blk_off+128],
                             u32t[g % 2].ap()[:, r0:r0+nr], start=True, stop=True)
                return ins

            @stage("vector")
            def _(v, g=g):
                return v.tensor_tensor(mrk[g % 2].ap()[:, :], w32tsb[g % 2].ap()[:, :],
                                       pR.ap()[:, :], A.mult)

            @stage("tensor")
            def _(t, g=g):
                qb4 = g % 2
                t.matmul(pTOP.ap()[64*qb4:64*qb4+4, :],
                         bass.AP(cs['s2top'], 4*g, [[512, 128], [1, 4]]), mrk[g % 2].ap()[:, :],
                         start=True, stop=True)
                return t.matmul(pBOT.ap()[64*qb4:64*qb4+4, :],
                         bass.AP(cs['s2bot'], 4*g, [[512, 128], [1, 4]]), mrk[g % 2].ap()[:, :],
                         start=True, stop=True)

            if g % 2 == 1:
                @stage("scalar")
                def _(act, g=g):
                    return act.activation(topsb.ap()[:, :], pTOP.ap()[:, :], ACTF.Copy,
                                          bias=0.0, scale=1.0)

                @stage("vector")
                def _(v, g=g):
                    return v.tensor_tensor(vbat.ap()[:, :], pBOT.ap()[:, :], topsb.ap()[:, :],
                                           A.subtract)

                @stage("vector")
                def _(v, g=g):
                    return v.tensor_tensor(vbat2.ap()[:, :], vbat.ap()[:, :],
                                           cs['fyconst'].ap()[:, :], A.mult)

                @stage("vector")
                def _(v, g=g, i=i):
                    vfr = v.alloc_register(f"vfb_{i}_{g}")
                    v.reg_load(vfr, metas.ap()[:1, 4*i+3:4*i+4])
                    with v.If_cmp(vfr, 0, "IS_EQ"):
                        v.tensor_tensor(vbat.ap()[:, :], vbat2.ap()[:, :], topsb.ap()[:, :],
                                        A.add)
                    with v.Else():
                        v.tensor_tensor(vbat.ap()[:, ::-1], vbat2.ap()[:, :], topsb.ap()[:, :],
                                        A.add)
                    v.end_ifs()
                    v.free_register(vfr)

                @stage("tensor")
                def _(t, g=g):
                    ins = None
                    for q in range(2):
                        for rc2 in range(4):
                            ins = t.transpose(pT.ap()[:, 16*q + 4*rc2: 16*q + 4*rc2 + 4],
                                              vbat.ap()[64*q:64*q+4, 128*rc2:128*(rc2+1)],
                                              cs['ident4'].ap()[64*q:64*q+4, :4])
                    return ins

                @stage("vector")
                def _(v, g=g):
                    g0 = g - 1
                    return v.tensor_copy(
                        bass.AP(ostg, 4*g0, [[2048, 128], [4, 2], [512, 4], [1, 4]]),
                        bass.AP(pT, 0, [[128, 128], [16, 2], [4, 4], [1, 4]]))

        # ---- output: DMA from ostg (vflip via DRAM-side row order) ----
        for s in range(4):
            @stage("sync")
            def _(sync, s=s, i=i):
                d = nc.alloc_semaphore(f"d_out_{i}_{s}")
                sync.dma_start(out[i][128*s:128*(s+1), :],
                               ostg.ap()[:, 512*s:512*(s+1)]).then_inc(d, 16)
                sync.wait_ge(d, 16)

    def emit_pt(i):
        @stage("sync")
        def _(sync, i=i):
            oy = sync.alloc_register(f"poy{i}")
            ox = sync.alloc_register(f"pox{i}")
            hf = sync.alloc_register(f"phf{i}")
            vf = sync.alloc_register(f"pvf{i}")
            t1 = sync.alloc_register(f"pt1{i}")
            sync.reg_load(oy, metas.ap()[:1, 4*i:4*i+1])
            sync.reg_load(ox, metas.ap()[:1, 4*i+1:4*i+2])
            sync.reg_load(hf, metas.ap()[:1, 4*i+2:4*i+3])
            sync.reg_load(vf, metas.ap()[:1, 4*i+3:4*i+4])
            with sync.If_cmp(vf, 0, "IS_NE"):
                sync.reg_mov(t1, 128)
                sync.reg_alu(t1, t1, oy, A.subtract)
                sync.reg_mov(oy, t1)
            sync.end_ifs()
            with sync.If_cmp(hf, 0, "IS_NE"):
                sync.reg_mov(t1, 128)
                sync.reg_alu(t1, t1, ox, A.subtract)
                sync.reg_mov(ox, t1)
            sync.end_ifs()
            soy = sync.snap(oy, donate=True)
            sox = sync.snap(ox, donate=True)
            d = nc.alloc_semaphore(f"d_pt_{i}")
            src2d = xs[i][bass.ds(soy, 128), bass.ds(sox, 512)]
            src3d = bass.AP(src2d.tensor, src2d.offset,
                            [[640, 128], [128*640, 4], [1, 512]],
                            runtime_checks=src2d.runtime_checks)
            sync.dma_start(bass.AP(xfull, 0, [[2048, 128], [512, 4], [1, 512]]),
                           src3d).then_inc(d, 16)
            sync.wait_ge(d, 16)
            for r in (oy, ox, hf, vf, t1):
                sync.free_register(r)
            try:
                sync.get_value_cache().clear()
            except Exception:
                pass

        # hflip into hstage (full sample), per slab
        for s in range(4):
            @stage("vector")
            def _(v, i=i, s=s):
                hfreg = v.alloc_register(f"phf_o_{i}_{s}")
                v.reg_load(hfreg, metas.ap()[:1, 4*i+2:4*i+3])
                with v.If_cmp(hfreg, 0, "IS_EQ"):
                    v.tensor_copy(hstage.ap()[:, 512*s:512*(s+1)],
                                  xfull.ap()[:, 512*s:512*(s+1)])
                with v.Else():
                    v.tensor_copy(bass.AP(hstage, 512*s + 511, [[2048, 128], [-1, 512]]),
                                  xfull.ap()[:, 512*s:512*(s+1)])
                v.end_ifs()
                v.free_register(hfreg)

        # vflip via transpose round-trip, one 128-col chunk q at a time:
        # hstage [128 rows, (s, col)] -> ctmp [128 cols(q), 512 rows (maybe reversed)]
        # -> outb [128 rows, (s, col)]
        for q in range(4):
            for s in range(4):
                @stage("tensor")
                def _(t, s=s, q=q):
                    t.transpose(pT.ap()[:, :],
                                bass.AP(hstage, 512*s + 128*q, [[2048, 128], [1, 128]]),
                                cs['identf32'].ap()[:, :])

                @stage("vector")
                def _(v, i=i, s=s, q=q):
                    vfr = v.alloc_register(f"pvt_{i}_{s}_{q}")
                    v.reg_load(vfr, metas.ap()[:1, 4*i+3:4*i+4])
                    with v.If_cmp(vfr, 0, "IS_EQ"):
                        v.tensor_copy(ctmp.ap()[:, 128*s:128*(s+1)], pT.ap()[:, :])
                    with v.Else():
                        v.tensor_copy(
                            bass.AP(ctmp, 511 - 128*s, [[512, 128], [-1, 128]]),
                            pT.ap()[:, :])
                    v.end_ifs()
                    v.free_register(vfr)

            for r in range(4):
                @stage("tensor")
                def _(t, r=r, q=q):
                    t.transpose(pT.ap()[:, :],
                                bass.AP(ctmp, 128*r, [[512, 128], [1, 128]]),
                                cs['identf32'].ap()[:, :])

                @stage("vector")
                def _(v, r=r, q=q):
                    v.tensor_copy(outb.ap()[:, 512*r + 128*q: 512*r + 128*(q+1)],
                                  pT.ap()[:, :])

        for s in range(4):
            @stage("sync")
            def _(sync, s=s, i=i):
                d = nc.alloc_semaphore(f"d_pto_{i}_{s}")
                sync.dma_start(out[i][128*s:128*(s+1), :],
                               outb.ap()[:, 512*s:512*s+512]).then_inc(d, 16)
                sync.wait_ge(d, 16)

    # ---- preamble: DMA consts + meta ----
    @stage("sync")
    def _(sync):
        d = nc.alloc_semaphore("d_pre")
        n = 0
        for name in consts:
            sync.dma_start(cs[name].ap()[:, :], cdram[name][:, :]).then_inc(d, 16)
            n += 16
        sync.dma_start(metas.ap()[:1, :], bass.AP(meta, 0, [[NS*4, 1], [1, NS*4]])).then_inc(d, 16)
        n += 16
        sync.wait_ge(d, n)

    for i in range(KC):
        emit_clahe(i)
    for i in range(KC, KC + KP):
        emit_pt(i)

    # ------------- emit engine programs -------------
    # Per-engine sems; each stage waits only on its immediate predecessor
    # stage (transitively ordering everything it needs), allowing engines
    # to overlap.
    esem = {e: nc.alloc_semaphore("seq_" + e) for e in
            ['sync', 'vector', 'scalar', 'tensor', 'gpsimd']}
    # precompute per-stage: (engine, fn, pred_engine, pred_ordinal)
    ordinal = {e: 0 for e in esem}
    plan = []
    prev = None
    for en, fn in stages:
        ordinal[en] += 1
        plan.append((en, fn, prev, ordinal[en]))
        prev = (en, ordinal[en])

    def emit_engine(eng, name):
        for en, fn, pred, ordn in plan:
            if en != name:
                continue
            if pred is not None:
                eng.wait_ge(esem[pred[0]], pred[1])
            _r = fn(eng)
            if _r is not None and hasattr(_r, 'then_inc'):
                _r.then_inc(esem[name], 1)
            else:
                eng.drain()
                eng.sem_inc(esem[name], 1)

    with nc.Block() as blk:
        @blk.sync
        def _(sync):
            emit_engine(sync, 'sync')

        @blk.vector
        def _(v):
            emit_engine(v, 'vector')

        @blk.scalar
        def _(act):
            emit_engine(act, 'scalar')

        @blk.tensor
        def _(t):
            emit_engine(t, 'tensor')

        @blk.gpsimd
        def _(g):
            emit_engine(g, 'gpsimd')

    return nc


# ======================== kernel() entry point ========================
import numpy as _np

_PROG_CACHE = {}

def _get_prog(KC, KP):
    key = (KC, KP)
    if key not in _PROG_CACHE:
        _PROG_CACHE[key] = build(KC, KP)
    return _PROG_CACHE[key]

def kernel(x, hflip, vflip, offy, offx, apply_clahe):
    from concourse.bass_utils import run_bass_kernel_spmd
    x = _np.asarray(x); B = x.shape[0]
    hflip = _np.asarray(hflip).astype(_np.int32)
    vflip = _np.asarray(vflip).astype(_np.int32)
    offy = _np.asarray(offy).astype(_np.int32)
    offx = _np.asarray(offx).astype(_np.int32)
    ac = _np.asarray(apply_clahe).astype(_np.int32)
    NCORES = 8
    cl = [i for i in range(B) if ac[i]]
    pt = [i for i in range(B) if not ac[i]]
    KC = (len(cl) + NCORES - 1) // NCORES
    KP = (len(pt) + NCORES - 1) // NCORES
    # assign round-robin
    slots = [[None] * (KC + KP) for _ in range(NCORES)]
    for j, i in enumerate(cl):
        slots[j % NCORES][j // NCORES] = i
    for j, i in enumerate(pt):
        slots[j % NCORES][KC + j // NCORES] = i
    consts = make_consts()
    nc = _get_prog(KC, KP)
    NS = KC + KP
    in_maps = []
    for c in range(NCORES):
        xs = _np.zeros((NS, 640, 640), _np.float32)
        meta = _np.zeros((NS, 4), _np.int32)
        for k in range(NS):
            i = slots[c][k]
            if i is None:
                i = 0  # padding: any sample; clahe-ness of slot fixed by position
            xs[k] = x[i, 0]
            meta[k] = (offy[i], offx[i], hflip[i], vflip[i])
        m = {"xs": xs, "meta": meta}
        for name, v in consts.items():
            m["c_" + name] = _np.ascontiguousarray(v.reshape(v.shape[0], -1))
        in_maps.append(m)
    res = run_bass_kernel_spmd(nc, in_maps, core_ids=list(range(NCORES)))
    out = _np.zeros((B, 1, 512, 512), _np.float32)
    for c in range(NCORES):
        o = res.results[c]["out"]
        for k in range(NS):
            i = slots[c][k]
            if i is not None:
                out[i, 0] = o[k]
    return out

